# revision 48
# baseline (speedup 1.0000x reference)
"""GCN encoder (2-layer graph conv) on 8 Trainium2 NeuronCores.

Sharding: nodes (rows of x / output) in contiguous 6272-row blocks across the
8 cores; edges partitioned by destination row; 256x256 weights replicated;
per-layer AllGather of the dense support into G=3 source-group tables (31/9/9
dest-tile split keeps table row ids int16; the last, small AllGather piece
gates the layer-2 transition); per-128-dest-tile segment-sum as scaled-one-hot
selector matmuls over dma_gathered source rows.

Perf structure (v1 baseline 1.82ms -> 1.37ms), in the order it was found:
  - v1's 106MB of host-built dense selector DMA is gone: selector chunks are
    built on device by DVE, one is_equal + one mult tensor_tensor per tile
    over broadcast APs of a constant iota row and per-lane (dest, val)
    columns (~0.9MB upload). Requires one edge per selector lane, so source
    dedup is dropped (+4% gather indices).
  - gather tables, rings, and AllGathers run in fp8e4: halves the dominant
    cost (452K 256B gather descriptors at ~21ns of DMA-engine time each) and
    the collective bytes. Measured rel_max 0.0153 (gate 2e-2); inputs are
    fixed-seed so this is deterministic.
  - DVE selector gen runs at 1 elem/cycle (broadcast operands disable the
    16-bit 2x mode) and paced the pipeline; every 3rd tile's selector is
    instead DMA-uploaded from a host-built table, balancing DVE against DMA
    slack.
  - rings must comfortably exceed the (PREF+1)-tile in-flight window: the
    tile framework's ring WAR tracking corrupts (not stalls) under overwrite
    pressure, and tight rings serialize gathers against PE consumption.
  - AllGather pieces fire as their dense/agg tiles complete, with a ring
    prefetch right after each so gathers start the moment a table lands.
Measured dead ends: G=(9,9,31) group flip (layer-2 transition then waits on
the big AG piece), separate L2 ring sets + early L2 prefetch, psA=3/psD=2
PSUM split, Pool-engine selector ops (compiler rejects is_equal on Pool).
"""

import os
import sys

if "/opt/trn_rl_repo" not in sys.path:
    sys.path.insert(0, "/opt/trn_rl_repo")

import numpy as np

import concourse.mybir as mybir
import concourse.tile as tile
from concourse import bacc, bass_utils
from concourse.bass import ts
from concourse.library_config import mlp

N = 50000
D = 256
NC = 8
P = 128
T = 49
SHARD = T * P  # 6272
NPAD = NC * SHARD  # 50176

G = 3
GTILES = (31, 9, 9)             # dest tiles per source group; the LAST AllGather
                                # piece must be small: it gates the layer-2
                                # transition (measured worse when flipped)
GT = (0, 31, 40, 49)            # tile boundaries
GB = (0, 31 * P, 40 * P, 49 * P)  # per-core row boundaries (3968, 5120, 6272)
GR = (31 * P, 9 * P, 9 * P)     # rows/core per group
TROWS = tuple(NC * r for r in GR)  # table rows: 31744, 9216, 9216 (< 32768)

RINGS = (96, 40, 40)  # ring slots (chunks) per source group; must comfortably
                      # exceed the in-flight window ((PREF+1) tiles of chunks +
                      # OPC overshoot) or gathers serialize against PE reads

# tiles whose selector chunks are DMA-uploaded (host-built) instead of
# DVE-generated: balances the DVE selector-gen pole against DMA slack.
UPLOAD_EVERY = 3
UP_TILES = tuple(t for t in range(T) if t % UPLOAD_EVERY == 0)
OPC = 8               # chunks per dma_gather op (1024 idxs)

F16 = mybir.dt.float16
F32 = mybir.dt.float32
I16 = mybir.dt.int16

# gather-table dtype: F16 (safe, rel_max ~3e-4) or float8e4 (halves gather/AG
# bytes; measured rel_max 0.0148 on the fixed inputs, under the 2e-2 gate).
TAB = mybir.dt.float8e4
TAB_BYTES = 2 if TAB == F16 else 1

_cache: dict = {}
last_results = None


def _build(cgt: tuple):
    """cgt: T-tuple of G-tuples of per-(tile, group) chunk counts."""
    key = cgt
    if key in _cache:
        return _cache[key]

    upcols = {}
    uc = 0
    for t in UP_TILES:
        upcols[t] = uc
        uc += sum(cgt[t])
    UPC = uc

    ct = [sum(c) for c in cgt]
    off = np.concatenate(([0], np.cumsum(ct))).astype(int)  # sel chunk offsets
    goff = []  # per-group cumulative chunk stream offsets over tiles
    for g in range(G):
        goff.append(
            np.concatenate(([0], np.cumsum([cgt[t][g] for t in range(T)]))).astype(int)
        )
    TOTC = int(off[-1])
    TOTG = [int(goff[g][-1]) for g in range(G)]
    TOT = sum(TOTG)
    GBASE = [8 * sum(TOTG[:g]) for g in range(G)]  # gidx column bases
    MAXCT = max(ct)

    nc = bacc.Bacc(
        "TRN2",
        target_bir_lowering=False,
        debug=False,
        num_devices=NC,
        num_swdge_queues=4,
    )

    xT_d = nc.dram_tensor("xT", [2, P, SHARD], F16, kind="ExternalInput")
    w1_d = nc.dram_tensor("W1h", [2, P, D], F16, kind="ExternalInput")
    w2_d = nc.dram_tensor("W2h", [2, P, D], F16, kind="ExternalInput")
    b1_d = nc.dram_tensor("b1c", [P, 2], F32, kind="ExternalInput")
    b2_d = nc.dram_tensor("b2b", [P, D], F32, kind="ExternalInput")
    gidx_d = nc.dram_tensor("gidx", [P, TOT * 8], I16, kind="ExternalInput")
    dst_d = nc.dram_tensor("dst", [P, TOTC], F16, kind="ExternalInput")
    val_d = nc.dram_tensor("val", [P, TOTC], F16, kind="ExternalInput")
    selu_d = nc.dram_tensor("selu", [P, UPC * P], F16, kind="ExternalInput")
    out_d = nc.dram_tensor("out", [SHARD, D], F32, kind="ExternalOutput")

    nc.gpsimd.load_library(mlp)

    rg = [list(range(NC))]

    with tile.TileContext(nc) as tc:
        with (
            tc.tile_pool(name="const", bufs=1) as const,
            tc.tile_pool(name="ring", bufs=1) as ring,
            tc.tile_pool(name="spool", bufs=4) as spool,
            tc.tile_pool(name="dense", bufs=3) as dense,
            tc.tile_pool(name="psD", bufs=3, space="PSUM") as psD,
            tc.tile_pool(name="psA", bufs=2, space="PSUM") as psA,
            tc.tile_pool(name="dram", bufs=1, space="DRAM") as dram,
        ):
            cc1 = [dram.tile([GR[g], D], TAB, name=f"cc1_{g}") for g in range(G)]
            t1 = [
                dram.tile([TROWS[g], D], TAB, addr_space="Shared", name=f"t1_{g}")
                for g in range(G)
            ]
            cc2 = [dram.tile([GR[g], D], TAB, name=f"cc2_{g}") for g in range(G)]
            t2 = [
                dram.tile([TROWS[g], D], TAB, addr_space="Shared", name=f"t2_{g}")
                for g in range(G)
            ]

            gidx = const.tile([P, TOT * 8], I16)
            nc.sync.dma_start(gidx[:], gidx_d[:])
            dst = const.tile([P, TOTC], F16)
            nc.sync.dma_start(dst[:], dst_d[:])
            val = const.tile([P, TOTC], F16)
            nc.sync.dma_start(val[:], val_d[:])
            b1 = const.tile([P, 2], F32)
            nc.sync.dma_start(b1[:], b1_d[:])
            b2 = const.tile([P, D], F32)
            nc.sync.dma_start(b2[:], b2_d[:])
            w1 = const.tile([P, 2 * D], F16)
            w2 = const.tile([P, 2 * D], F16)
            xsb = const.tile([P, 2 * SHARD], F16, tag="xht", name="xsb")
            hT = const.tile([P, 2 * SHARD], F16, tag="xht", name="hT")
            for h in range(2):
                nc.sync.dma_start(w1[:, h * D : (h + 1) * D], w1_d[h])
                nc.sync.dma_start(w2[:, h * D : (h + 1) * D], w2_d[h])
                # load x in 7-tile pieces so dense tile 0 starts immediately
                for q in range(7):
                    nc.sync.dma_start(
                        xsb[:, h * SHARD + q * 7 * P : h * SHARD + (q + 1) * 7 * P],
                        xT_d[h, :, q * 7 * P : (q + 1) * 7 * P],
                    )

            iot = const.tile([P, P], F16, name="iota")
            nc.gpsimd.iota(
                iot[:],
                [[1, P]],
                channel_multiplier=0,
                allow_small_or_imprecise_dtypes=True,
            )

            rings = [
                ring.tile([P, RINGS[g], D], TAB, name=f"ring{g}") for g in range(G)
            ]

            # per-tile matmul consumption map: k -> (group, chunk-in-stream)
            kmap = []
            for t in range(T):
                m = []
                for g in range(G):
                    for j in range(cgt[t][g]):
                        m.append((g, int(goff[g][t]) + j))
                kmap.append(m)

            def dense_tile(src_sb, w_sb, cc, t):
                ps = psD.tile([P, D], F32, tag="psD", name="ps_dense")
                for h in range(2):
                    nc.tensor.matmul(
                        ps,
                        lhsT=src_sb[:, h * SHARD + t * P : h * SHARD + (t + 1) * P],
                        rhs=w_sb[:, h * D : (h + 1) * D],
                        start=(h == 0),
                        stop=(h == 1),
                    )
                st = dense.tile([P, D], TAB, tag="stage", name="stage")
                nc.scalar.copy(st[:], ps[:])
                g = next(gg for gg in range(G) if GT[gg] <= t < GT[gg + 1])
                nc.sync.dma_start(cc[g][ts(t - GT[g], P), :], st[:])

            def ag(cc_g, t_out):
                nc.gpsimd.collective_compute(
                    "AllGather",
                    mybir.AluOpType.bypass,
                    replica_groups=rg,
                    ins=[cc_g.opt()],
                    outs=[t_out.opt()],
                )

            qctr = [0]

            def make_streams(tabs):
                state = [
                    {"next": 0, "tot": TOTG[g], "table": tabs[g], "ring": rings[g],
                     "base": GBASE[g], "rsz": RINGS[g]}
                    for g in range(G)
                ]

                def ensure(g, upto):
                    s = state[g]
                    while s["next"] < min(upto, s["tot"]):
                        a = s["next"]
                        n = min(OPC, s["tot"] - a)
                        nc.gpsimd.dma_gather(
                            s["ring"][:, a % s["rsz"] : a % s["rsz"] + n, :],
                            s["table"][:],
                            gidx[:, s["base"] + a * 8 : s["base"] + (a + n) * 8],
                            num_idxs=n * P,
                            num_idxs_reg=n * P,
                            elem_size=D,
                            queue_num=qctr[0] % 4,
                        )
                        qctr[0] += 1
                        s["next"] = a + n
                return ensure

            PREF = 2  # prefetch horizon in tiles

            def sel_tile(t):
                # selector chunks run at 1 elem/cycle on DVE (broadcast
                # operands disable the 16-bit 2x mode; Pool rejects is_equal
                # TensorTensor), so DVE generation is the pacing engine during
                # agg. UP_TILES instead DMA a host-built selector (DMA has
                # slack), balancing the two engines.
                c = ct[t]
                o = int(off[t])
                s = spool.tile([P, MAXCT, P], F16, tag="sel", name="sel")
                if t in upcols:
                    u = upcols[t]
                    nc.sync.dma_start(
                        s[:, :c, :],
                        selu_d[:, u * P : (u + c) * P].rearrange(
                            "p (c q) -> p c q", c=c
                        ),
                    )
                    return s
                nc.vector.tensor_tensor(
                    s[:, :c, :],
                    iot[:, None, :].broadcast_to([P, c, P]),
                    dst[:, o : o + c, None].broadcast_to([P, c, P]),
                    mybir.AluOpType.is_equal,
                )
                nc.vector.tensor_tensor(
                    s[:, :c, :],
                    s[:, :c, :],
                    val[:, o : o + c, None].broadcast_to([P, c, P]),
                    mybir.AluOpType.mult,
                )
                return s

            # ---------- layer 1 dense (+ pipelined AllGather pieces) -------
            ensure1 = make_streams(t1)
            for t in range(T):
                dense_tile(xsb, w1, cc1, t)
                for g in range(G):
                    if t == GT[g + 1] - 1:
                        ag(cc1[g], t1[g])
                        ensure1(g, RINGS[g])  # prefetch as soon as table lands

            # ---------- layer 1 agg (+ interleaved layer-2 dense) ----------
            for t in range(T):
                s = sel_tile(t)
                tp = min(t + PREF, T - 1)
                for g in range(G):
                    ensure1(g, int(goff[g][tp + 1]))
                pss = [
                    psA.tile([P, P], F32, tag=f"agg{h}", name=f"agg{h}")
                    for h in range(2)
                ]
                c = ct[t]
                for k, (g, j) in enumerate(kmap[t]):
                    sl = j % RINGS[g]
                    for h in range(2):
                        nc.tensor.matmul(
                            pss[h],
                            lhsT=rings[g][:, sl, h * P : (h + 1) * P],
                            rhs=s[:, k, :],
                            start=(k == 0),
                            stop=(k == c - 1),
                        )
                for h in range(2):
                    nc.scalar.activation(
                        hT[:, h * SHARD + t * P : h * SHARD + (t + 1) * P],
                        pss[h][:],
                        mybir.ActivationFunctionType.Relu,
                        bias=b1[:, h : h + 1],
                    )
                dense_tile(hT, w2, cc2, t)
                for g in range(G):
                    if t == GT[g + 1] - 1:
                        ag(cc2[g], t2[g])
            # ---------- layer 2 ----------
            ensure2 = make_streams(t2)
            for t in range(T):
                s = sel_tile(t)
                tp = min(t + PREF, T - 1)
                for g in range(G):
                    ensure2(g, int(goff[g][tp + 1]))
                ps = psD.tile([P, D], F32, tag="psD", name="agg2")
                c = ct[t]
                for k, (g, j) in enumerate(kmap[t]):
                    sl = j % RINGS[g]
                    nc.tensor.matmul(
                        ps,
                        lhsT=s[:, k, :],
                        rhs=rings[g][:, sl, :],
                        start=(k == 0),
                        stop=(k == c - 1),
                    )
                nc.vector.tensor_tensor(ps[:], ps[:], b2[:], mybir.AluOpType.add)
                ot = dense.tile([P, D], F32, tag="ot", name="ot")
                nc.scalar.activation(ot[:], ps[:], mybir.ActivationFunctionType.Relu)
                nc.sync.dma_start(out_d[ts(t, P), :], ot[:])

    nc.compile()
    _cache[key] = nc
    return nc


def _wrap_idx16(flat: np.ndarray) -> np.ndarray:
    """[L] int -> [128, L/16] int16 SBUF wrap: sb[p, s] = flat[s*16 + p%16]."""
    L = flat.shape[0]
    base = flat.reshape(L // 16, 16).T.astype(np.int16)
    return np.tile(base, (8, 1))


def _preprocess(adj_rows, adj_cols, adj_vals):
    r = np.asarray(adj_rows).astype(np.int64)
    c = np.asarray(adj_cols).astype(np.int64)
    v = np.asarray(adj_vals).astype(np.float32)

    core = r // SHARD
    tile_id = (r % SHARD) // P
    dest_local = r % P
    s_core = c // SHARD
    s_loc = c % SHARD
    grp = (s_loc >= GB[1]).astype(np.int64) + (s_loc >= GB[2]).astype(np.int64)
    idx_local = s_core * np.asarray(GR)[grp] + (s_loc - np.asarray(GB)[grp])

    # slot assignment: rank within each (core, tile, group), edge order
    key = ((core * T + tile_id) * G + grp)
    order = np.argsort(key, kind="stable")
    key_s = key[order]
    first = np.ones(order.shape[0], bool)
    first[1:] = key_s[1:] != key_s[:-1]
    idx = np.arange(order.shape[0])
    start = idx[first]
    gid = np.cumsum(first) - 1
    slot = idx - start[gid]

    cnt = np.zeros(NC * T * G, np.int64)
    np.add.at(cnt, key, 1)
    cnt = cnt.reshape(NC, T, G)
    cgt = tuple(
        tuple(max(1, -(-int(cnt[:, t, g].max()) // P)) for g in range(G))
        for t in range(T)
    )

    ct = [sum(cg) for cg in cgt]
    off = np.concatenate(([0], np.cumsum(ct))).astype(int)
    goff = []
    for g in range(G):
        goff.append(
            np.concatenate(([0], np.cumsum([cgt[t][g] for t in range(T)]))).astype(int)
        )
    TOTC = int(off[-1])
    TOTG = [int(goff[g][-1]) for g in range(G)]
    TOT = sum(TOTG)

    core_s = core[order]
    tile_s = tile_id[order]
    grp_s = grp[order]
    dest_s = dest_local[order]
    v_s = v[order]
    idx_s = idx_local[order]

    chunk_in_tg = slot // P
    lane = slot % P
    # sel column: off[t] + chunks of lower groups + chunk_in_tg
    gprefix = np.zeros((T, G), np.int64)
    for t in range(T):
        acc = 0
        for g in range(G):
            gprefix[t, g] = acc
            acc += cgt[t][g]
    scol = off[tile_s] + gprefix[tile_s, grp_s] + chunk_in_tg

    dst = np.zeros((NC, P, TOTC), np.float16)
    valq = np.zeros((NC, P, TOTC), np.float16)
    dst[core_s, lane, scol] = dest_s.astype(np.float16)
    valq[core_s, lane, scol] = v_s.astype(np.float16)

    # gather index streams, group-concatenated
    stream_base = np.asarray([sum(TOTG[:g]) for g in range(G)])
    stream_chunk = stream_base[grp_s] + goff_lookup(goff, tile_s, grp_s) + chunk_in_tg
    idx_pad = np.zeros((NC, TOT, P), np.int16)
    idx_pad[core_s, stream_chunk, lane] = idx_s.astype(np.int16)
    gidx = np.zeros((NC, P, TOT * 8), np.int16)
    for cr in range(NC):
        gidx[cr] = _wrap_idx16(idx_pad[cr].reshape(-1))

    return cgt, gidx, dst, valq


def goff_lookup(goff, tile_s, grp_s):
    tab = np.stack([goff[g][:T] for g in range(G)], axis=1)  # [T, G]
    return tab[tile_s, grp_s]


def kernel(
    x, adj_rows, adj_cols, adj_vals, pad_n, pos_idx, W1, b1, W2, b2
) -> np.ndarray:
    x = np.asarray(x, np.float32)
    W1 = np.asarray(W1, np.float32)
    b1 = np.asarray(b1, np.float32)
    W2 = np.asarray(W2, np.float32)
    b2 = np.asarray(b2, np.float32)
    pos_idx = np.asarray(pos_idx).astype(np.int64)
    pad_n_i = int(pad_n)
    assert x.shape == (N, D)

    cgt, gidx, dstq, valq = _preprocess(adj_rows, adj_cols, adj_vals)
    nc = _build(cgt)

    # host-built dense selector for the uploaded tiles
    ct = [sum(cg) for cg in cgt]
    off = np.concatenate(([0], np.cumsum(ct))).astype(int)
    UPC = sum(ct[t] for t in UP_TILES)
    selu = np.zeros((NC, P, UPC, P), np.float16)
    u = 0
    for t in UP_TILES:
        c = ct[t]
        o = int(off[t])
        d = dstq[:, :, o : o + c].astype(np.int64)
        v = valq[:, :, o : o + c]
        selu[
            np.arange(NC)[:, None, None],
            np.arange(P)[None, :, None],
            u + np.arange(c)[None, None, :],
            d,
        ] = v
        u += c
    selu = selu.reshape(NC, P, UPC * P)

    xpad = np.zeros((NPAD, D), np.float32)
    xpad[:N] = x
    w1h = W1.astype(np.float16).reshape(2, P, D)
    w2h = W2.astype(np.float16).reshape(2, P, D)
    b1c = np.ascontiguousarray(b1.reshape(2, P).T.astype(np.float32))
    b2b = np.ascontiguousarray(np.broadcast_to(b2, (P, D)).astype(np.float32))

    in_maps = []
    for cr in range(NC):
        xT = np.ascontiguousarray(
            xpad[cr * SHARD : (cr + 1) * SHARD].T.astype(np.float16).reshape(2, P, SHARD)
        )
        in_maps.append(
            {
                "xT": xT,
                "W1h": w1h,
                "W2h": w2h,
                "b1c": b1c,
                "b2b": b2b,
                "gidx": np.ascontiguousarray(gidx[cr]),
                "dst": np.ascontiguousarray(dstq[cr]),
                "val": np.ascontiguousarray(valq[cr]),
                "selu": np.ascontiguousarray(selu[cr]),
            }
        )

    trace = bool(int(os.environ.get("KERNEL_TRACE", "0")))
    res = None
    for attempt in range(3):
        try:
            res = bass_utils.run_bass_kernel_spmd(
                nc, in_maps, core_ids=list(range(NC)), trace=trace
            )
            break
        except Exception:
            if attempt == 2:
                raise
            import time as _time

            _time.sleep(10.0)
    global last_results
    last_results = res

    h2 = np.concatenate([res.results[cr]["out"] for cr in range(NC)], axis=0)[:N]
    out = np.zeros((pad_n_i, D), np.float32)
    out[pos_idx] = h2
    return out


# revision 50
# speedup vs baseline: 1.0071x; 1.0071x over previous
"""GCN encoder (2-layer graph conv) on 8 Trainium2 NeuronCores.

Sharding: nodes (rows of x / output) in contiguous 6272-row blocks across the
8 cores; edges partitioned by destination row; 256x256 weights replicated;
per-layer AllGather of the dense support into G=3 source-group tables (31/9/9
dest-tile split keeps table row ids int16; the last, small AllGather piece
gates the layer-2 transition); per-128-dest-tile segment-sum as scaled-one-hot
selector matmuls over dma_gathered source rows.

Perf structure (v1 baseline 1.82ms -> 1.37ms), in the order it was found:
  - v1's 106MB of host-built dense selector DMA is gone: selector chunks are
    built on device by DVE, one is_equal + one mult tensor_tensor per tile
    over broadcast APs of a constant iota row and per-lane (dest, val)
    columns (~0.9MB upload). Requires one edge per selector lane, so source
    dedup is dropped (+4% gather indices).
  - gather tables, rings, and AllGathers run in fp8e4: halves the dominant
    cost (452K 256B gather descriptors at ~21ns of DMA-engine time each) and
    the collective bytes. Measured rel_max 0.0153 (gate 2e-2); inputs are
    fixed-seed so this is deterministic.
  - DVE selector gen runs at 1 elem/cycle (broadcast operands disable the
    16-bit 2x mode) and paced the pipeline; every 3rd tile's selector is
    instead DMA-uploaded from a host-built table, balancing DVE against DMA
    slack.
  - rings must comfortably exceed the (PREF+1)-tile in-flight window: the
    tile framework's ring WAR tracking corrupts (not stalls) under overwrite
    pressure, and tight rings serialize gathers against PE consumption.
  - AllGather pieces fire as their dense/agg tiles complete, with a ring
    prefetch right after each so gathers start the moment a table lands.
Measured dead ends: G=(9,9,31) group flip (layer-2 transition then waits on
the big AG piece), separate L2 ring sets + early L2 prefetch, psA=3/psD=2
PSUM split, Pool-engine selector ops (compiler rejects is_equal on Pool).
"""

import os
import sys

if "/opt/trn_rl_repo" not in sys.path:
    sys.path.insert(0, "/opt/trn_rl_repo")

import numpy as np

import concourse.mybir as mybir
import concourse.tile as tile
from concourse import bacc, bass_utils
from concourse.bass import ts
from concourse.library_config import mlp

N = 50000
D = 256
NC = 8
P = 128
T = 49
SHARD = T * P  # 6272
NPAD = NC * SHARD  # 50176

G = 3
GTILES = (31, 9, 9)             # dest tiles per source group; the LAST AllGather
                                # piece must be small: it gates the layer-2
                                # transition (measured worse when flipped)
GT = (0, 31, 40, 49)            # tile boundaries
GB = (0, 31 * P, 40 * P, 49 * P)  # per-core row boundaries (3968, 5120, 6272)
GR = (31 * P, 9 * P, 9 * P)     # rows/core per group
TROWS = tuple(NC * r for r in GR)  # table rows: 31744, 9216, 9216 (< 32768)

RINGS = (96, 40, 40)  # ring slots (chunks) per source group; must comfortably
                      # exceed the in-flight window ((PREF+1) tiles of chunks +
                      # OPC overshoot) or gathers serialize against PE reads

# tiles whose selector chunks are DMA-uploaded (host-built) instead of
# DVE-generated: balances the DVE selector-gen pole against DMA slack.
UPLOAD_EVERY = 3
UP_TILES = tuple(t for t in range(T) if t % UPLOAD_EVERY == 0)
OPC = 8               # chunks per dma_gather op (1024 idxs)

F16 = mybir.dt.float16
F32 = mybir.dt.float32
I16 = mybir.dt.int16

# gather-table dtype: F16 (safe, rel_max ~3e-4) or float8e4 (halves gather/AG
# bytes; measured rel_max 0.0148 on the fixed inputs, under the 2e-2 gate).
TAB = mybir.dt.float8e4
TAB_BYTES = 2 if TAB == F16 else 1

_cache: dict = {}
last_results = None


def _build(cgt: tuple):
    """cgt: T-tuple of G-tuples of per-(tile, group) chunk counts."""
    key = cgt
    if key in _cache:
        return _cache[key]

    upcols = {}
    uc = 0
    for t in UP_TILES:
        upcols[t] = uc
        uc += sum(cgt[t])
    UPC = uc

    ct = [sum(c) for c in cgt]
    off = np.concatenate(([0], np.cumsum(ct))).astype(int)  # sel chunk offsets
    goff = []  # per-group cumulative chunk stream offsets over tiles
    for g in range(G):
        goff.append(
            np.concatenate(([0], np.cumsum([cgt[t][g] for t in range(T)]))).astype(int)
        )
    TOTC = int(off[-1])
    TOTG = [int(goff[g][-1]) for g in range(G)]
    TOT = sum(TOTG)
    GBASE = [8 * sum(TOTG[:g]) for g in range(G)]  # gidx column bases
    MAXCT = max(ct)

    nc = bacc.Bacc(
        "TRN2",
        target_bir_lowering=False,
        debug=False,
        num_devices=NC,
        num_swdge_queues=4,
    )

    xT_d = nc.dram_tensor("xT", [2, P, SHARD], F16, kind="ExternalInput")
    w1_d = nc.dram_tensor("W1h", [2, P, D], F16, kind="ExternalInput")
    w2_d = nc.dram_tensor("W2h", [2, P, D], F16, kind="ExternalInput")
    b1_d = nc.dram_tensor("b1c", [P, 2], F32, kind="ExternalInput")
    b2_d = nc.dram_tensor("b2b", [P, D], F32, kind="ExternalInput")
    gidx_d = nc.dram_tensor("gidx", [P, TOT * 8], I16, kind="ExternalInput")
    dst_d = nc.dram_tensor("dst", [P, TOTC], F16, kind="ExternalInput")
    val_d = nc.dram_tensor("val", [P, TOTC], F16, kind="ExternalInput")
    selu_d = nc.dram_tensor("selu", [P, UPC * P], F16, kind="ExternalInput")
    out_d = nc.dram_tensor("out", [SHARD, D], F32, kind="ExternalOutput")

    nc.gpsimd.load_library(mlp)

    rg = [list(range(NC))]

    with tile.TileContext(nc) as tc:
        with (
            tc.tile_pool(name="const", bufs=1) as const,
            tc.tile_pool(name="ring", bufs=1) as ring,
            tc.tile_pool(name="spool", bufs=4) as spool,
            tc.tile_pool(name="dense", bufs=3) as dense,
            tc.tile_pool(name="psD", bufs=3, space="PSUM") as psD,
            tc.tile_pool(name="psA", bufs=2, space="PSUM") as psA,
            tc.tile_pool(name="dram", bufs=1, space="DRAM") as dram,
        ):
            cc1 = [dram.tile([GR[g], D], TAB, name=f"cc1_{g}") for g in range(G)]
            t1 = [
                dram.tile([TROWS[g], D], TAB, addr_space="Shared", name=f"t1_{g}")
                for g in range(G)
            ]
            cc2 = [dram.tile([GR[g], D], TAB, name=f"cc2_{g}") for g in range(G)]
            t2 = [
                dram.tile([TROWS[g], D], TAB, addr_space="Shared", name=f"t2_{g}")
                for g in range(G)
            ]

            gidx = const.tile([P, TOT * 8], I16)
            nc.sync.dma_start(gidx[:], gidx_d[:])
            dst = const.tile([P, TOTC], F16)
            nc.sync.dma_start(dst[:], dst_d[:])
            val = const.tile([P, TOTC], F16)
            nc.sync.dma_start(val[:], val_d[:])
            b1 = const.tile([P, 2], F32)
            nc.sync.dma_start(b1[:], b1_d[:])
            b2 = const.tile([P, D], F32)
            nc.sync.dma_start(b2[:], b2_d[:])
            w1 = const.tile([P, 2 * D], F16)
            w2 = const.tile([P, 2 * D], F16)
            xsb = const.tile([P, 2 * SHARD], F16, tag="xht", name="xsb")
            hT = const.tile([P, 2 * SHARD], F16, tag="xht", name="hT")
            for h in range(2):
                nc.sync.dma_start(w1[:, h * D : (h + 1) * D], w1_d[h])
                nc.sync.dma_start(w2[:, h * D : (h + 1) * D], w2_d[h])
                # load x in 7-tile pieces so dense tile 0 starts immediately
                for q in range(7):
                    nc.sync.dma_start(
                        xsb[:, h * SHARD + q * 7 * P : h * SHARD + (q + 1) * 7 * P],
                        xT_d[h, :, q * 7 * P : (q + 1) * 7 * P],
                    )

            iot = const.tile([P, P], F16, name="iota")
            nc.gpsimd.iota(
                iot[:],
                [[1, P]],
                channel_multiplier=0,
                allow_small_or_imprecise_dtypes=True,
            )

            rings = [
                ring.tile([P, RINGS[g], D], TAB, name=f"ring{g}") for g in range(G)
            ]

            # per-tile matmul consumption map: k -> (group, chunk-in-stream)
            kmap = []
            for t in range(T):
                m = []
                for g in range(G):
                    for j in range(cgt[t][g]):
                        m.append((g, int(goff[g][t]) + j))
                kmap.append(m)

            def dense_tile(src_sb, w_sb, cc, t):
                ps = psD.tile([P, D], F32, tag="psD", name="ps_dense")
                for h in range(2):
                    nc.tensor.matmul(
                        ps,
                        lhsT=src_sb[:, h * SHARD + t * P : h * SHARD + (t + 1) * P],
                        rhs=w_sb[:, h * D : (h + 1) * D],
                        start=(h == 0),
                        stop=(h == 1),
                    )
                st = dense.tile([P, D], TAB, tag="stage", name="stage")
                nc.scalar.copy(st[:], ps[:])
                g = next(gg for gg in range(G) if GT[gg] <= t < GT[gg + 1])
                nc.sync.dma_start(cc[g][ts(t - GT[g], P), :], st[:])

            def ag(cc_g, t_out):
                nc.gpsimd.collective_compute(
                    "AllGather",
                    mybir.AluOpType.bypass,
                    replica_groups=rg,
                    ins=[cc_g.opt()],
                    outs=[t_out.opt()],
                )

            qctr = [0]

            def make_streams(tabs):
                state = [
                    {"next": 0, "tot": TOTG[g], "table": tabs[g], "ring": rings[g],
                     "base": GBASE[g], "rsz": RINGS[g]}
                    for g in range(G)
                ]

                def ensure(g, upto):
                    s = state[g]
                    while s["next"] < min(upto, s["tot"]):
                        a = s["next"]
                        n = min(OPC, s["tot"] - a)
                        nc.gpsimd.dma_gather(
                            s["ring"][:, a % s["rsz"] : a % s["rsz"] + n, :],
                            s["table"][:],
                            gidx[:, s["base"] + a * 8 : s["base"] + (a + n) * 8],
                            num_idxs=n * P,
                            num_idxs_reg=n * P,
                            elem_size=D,
                            queue_num=qctr[0] % 4,
                        )
                        qctr[0] += 1
                        s["next"] = a + n
                return ensure

            PREF = 2  # prefetch horizon in tiles

            def sel_tile(t):
                # selector chunks run at 1 elem/cycle on DVE (broadcast
                # operands disable the 16-bit 2x mode; Pool rejects is_equal
                # TensorTensor), so DVE generation is the pacing engine during
                # agg. UP_TILES instead DMA a host-built selector (DMA has
                # slack), balancing the two engines.
                c = ct[t]
                o = int(off[t])
                s = spool.tile([P, MAXCT, P], F16, tag="sel", name="sel")
                if t in upcols:
                    u = upcols[t]
                    nc.sync.dma_start(
                        s[:, :c, :],
                        selu_d[:, u * P : (u + c) * P].rearrange(
                            "p (c q) -> p c q", c=c
                        ),
                    )
                    return s
                nc.vector.tensor_tensor(
                    s[:, :c, :],
                    iot[:, None, :].broadcast_to([P, c, P]),
                    dst[:, o : o + c, None].broadcast_to([P, c, P]),
                    mybir.AluOpType.is_equal,
                )
                nc.vector.tensor_tensor(
                    s[:, :c, :],
                    s[:, :c, :],
                    val[:, o : o + c, None].broadcast_to([P, c, P]),
                    mybir.AluOpType.mult,
                )
                return s

            # ---------- layer 1 dense (+ pipelined AllGather pieces) -------
            ensure1 = make_streams(t1)
            for t in range(T):
                dense_tile(xsb, w1, cc1, t)
                for g in range(G):
                    if t == GT[g + 1] - 1:
                        ag(cc1[g], t1[g])
                        ensure1(g, RINGS[g])  # prefetch as soon as table lands

            # ---------- layer 1 agg (+ interleaved layer-2 dense) ----------
            for t in range(T):
                s = sel_tile(t)
                tp = min(t + PREF, T - 1)
                for g in range(G):
                    ensure1(g, int(goff[g][tp + 1]))
                pss = [
                    psA.tile([P, P], F32, tag=f"agg{h}", name=f"agg{h}")
                    for h in range(2)
                ]
                c = ct[t]
                for k, (g, j) in enumerate(kmap[t]):
                    sl = j % RINGS[g]
                    for h in range(2):
                        nc.tensor.matmul(
                            pss[h],
                            lhsT=rings[g][:, sl, h * P : (h + 1) * P],
                            rhs=s[:, k, :],
                            start=(k == 0),
                            stop=(k == c - 1),
                        )
                for h in range(2):
                    nc.scalar.activation(
                        hT[:, h * SHARD + t * P : h * SHARD + (t + 1) * P],
                        pss[h][:],
                        mybir.ActivationFunctionType.Relu,
                        bias=b1[:, h : h + 1],
                    )
                dense_tile(hT, w2, cc2, t)
                for g in range(G):
                    if t == GT[g + 1] - 1:
                        ag(cc2[g], t2[g])
            # ---------- layer 2 ----------
            ensure2 = make_streams(t2)
            for t in range(T):
                s = sel_tile(t)
                tp = min(t + PREF, T - 1)
                for g in range(G):
                    ensure2(g, int(goff[g][tp + 1]))
                ps = psD.tile([P, D], F32, tag="psD", name="agg2")
                c = ct[t]
                for k, (g, j) in enumerate(kmap[t]):
                    sl = j % RINGS[g]
                    nc.tensor.matmul(
                        ps,
                        lhsT=s[:, k, :],
                        rhs=rings[g][:, sl, :],
                        start=(k == 0),
                        stop=(k == c - 1),
                    )
                nc.vector.tensor_tensor(ps[:], ps[:], b2[:], mybir.AluOpType.add)
                ot = dense.tile([P, D], F32, tag="ot", name="ot")
                nc.scalar.activation(ot[:], ps[:], mybir.ActivationFunctionType.Relu)
                nc.sync.dma_start(out_d[ts(t, P), :], ot[:])

    nc.compile()
    _cache[key] = nc
    return nc


def _wrap_idx16(flat: np.ndarray) -> np.ndarray:
    """[L] int -> [128, L/16] int16 SBUF wrap: sb[p, s] = flat[s*16 + p%16]."""
    L = flat.shape[0]
    base = flat.reshape(L // 16, 16).T.astype(np.int16)
    return np.tile(base, (8, 1))


def _preprocess(adj_rows, adj_cols, adj_vals):
    r = np.asarray(adj_rows).astype(np.int64)
    c = np.asarray(adj_cols).astype(np.int64)
    v = np.asarray(adj_vals).astype(np.float32)

    core = r // SHARD
    tile_id = (r % SHARD) // P
    dest_local = r % P
    s_core = c // SHARD
    s_loc = c % SHARD
    grp = (s_loc >= GB[1]).astype(np.int64) + (s_loc >= GB[2]).astype(np.int64)
    idx_local = s_core * np.asarray(GR)[grp] + (s_loc - np.asarray(GB)[grp])

    # slot assignment: rank within each (core, tile, group), edge order
    key = ((core * T + tile_id) * G + grp)
    order = np.argsort(key, kind="stable")
    key_s = key[order]
    first = np.ones(order.shape[0], bool)
    first[1:] = key_s[1:] != key_s[:-1]
    idx = np.arange(order.shape[0])
    start = idx[first]
    gid = np.cumsum(first) - 1
    slot = idx - start[gid]

    cnt = np.zeros(NC * T * G, np.int64)
    np.add.at(cnt, key, 1)
    cnt = cnt.reshape(NC, T, G)
    cgt = tuple(
        tuple(max(1, -(-int(cnt[:, t, g].max()) // P)) for g in range(G))
        for t in range(T)
    )

    ct = [sum(cg) for cg in cgt]
    off = np.concatenate(([0], np.cumsum(ct))).astype(int)
    goff = []
    for g in range(G):
        goff.append(
            np.concatenate(([0], np.cumsum([cgt[t][g] for t in range(T)]))).astype(int)
        )
    TOTC = int(off[-1])
    TOTG = [int(goff[g][-1]) for g in range(G)]
    TOT = sum(TOTG)

    core_s = core[order]
    tile_s = tile_id[order]
    grp_s = grp[order]
    dest_s = dest_local[order]
    v_s = v[order]
    idx_s = idx_local[order]

    chunk_in_tg = slot // P
    lane = slot % P
    # sel column: off[t] + chunks of lower groups + chunk_in_tg
    gprefix = np.zeros((T, G), np.int64)
    for t in range(T):
        acc = 0
        for g in range(G):
            gprefix[t, g] = acc
            acc += cgt[t][g]
    scol = off[tile_s] + gprefix[tile_s, grp_s] + chunk_in_tg

    dst = np.zeros((NC, P, TOTC), np.float16)
    valq = np.zeros((NC, P, TOTC), np.float16)
    dst[core_s, lane, scol] = dest_s.astype(np.float16)
    valq[core_s, lane, scol] = v_s.astype(np.float16)

    # gather index streams, group-concatenated
    stream_base = np.asarray([sum(TOTG[:g]) for g in range(G)])
    stream_chunk = stream_base[grp_s] + goff_lookup(goff, tile_s, grp_s) + chunk_in_tg
    idx_pad = np.zeros((NC, TOT, P), np.int16)
    idx_pad[core_s, stream_chunk, lane] = idx_s.astype(np.int16)
    gidx = np.zeros((NC, P, TOT * 8), np.int16)
    for cr in range(NC):
        gidx[cr] = _wrap_idx16(idx_pad[cr].reshape(-1))

    return cgt, gidx, dst, valq


def goff_lookup(goff, tile_s, grp_s):
    tab = np.stack([goff[g][:T] for g in range(G)], axis=1)  # [T, G]
    return tab[tile_s, grp_s]


def kernel(
    x, adj_rows, adj_cols, adj_vals, pad_n, pos_idx, W1, b1, W2, b2
) -> np.ndarray:
    x = np.asarray(x, np.float32)
    W1 = np.asarray(W1, np.float32)
    b1 = np.asarray(b1, np.float32)
    W2 = np.asarray(W2, np.float32)
    b2 = np.asarray(b2, np.float32)
    pos_idx = np.asarray(pos_idx).astype(np.int64)
    pad_n_i = int(pad_n)
    assert x.shape == (N, D)

    cgt, gidx, dstq, valq = _preprocess(adj_rows, adj_cols, adj_vals)
    nc = _build(cgt)

    # host-built dense selector for the uploaded tiles
    ct = [sum(cg) for cg in cgt]
    off = np.concatenate(([0], np.cumsum(ct))).astype(int)
    UPC = sum(ct[t] for t in UP_TILES)
    selu = np.zeros((NC, P, UPC, P), np.float16)
    u = 0
    for t in UP_TILES:
        c = ct[t]
        o = int(off[t])
        d = dstq[:, :, o : o + c].astype(np.int64)
        v = valq[:, :, o : o + c]
        selu[
            np.arange(NC)[:, None, None],
            np.arange(P)[None, :, None],
            u + np.arange(c)[None, None, :],
            d,
        ] = v
        u += c
    selu = selu.reshape(NC, P, UPC * P)

    xpad = np.zeros((NPAD, D), np.float32)
    xpad[:N] = x
    w1h = W1.astype(np.float16).reshape(2, P, D)
    w2h = W2.astype(np.float16).reshape(2, P, D)
    b1c = np.ascontiguousarray(b1.reshape(2, P).T.astype(np.float32))
    b2b = np.ascontiguousarray(np.broadcast_to(b2, (P, D)).astype(np.float32))

    in_maps = []
    for cr in range(NC):
        xT = np.ascontiguousarray(
            xpad[cr * SHARD : (cr + 1) * SHARD].T.astype(np.float16).reshape(2, P, SHARD)
        )
        in_maps.append(
            {
                "xT": xT,
                "W1h": w1h,
                "W2h": w2h,
                "b1c": b1c,
                "b2b": b2b,
                "gidx": np.ascontiguousarray(gidx[cr]),
                "dst": np.ascontiguousarray(dstq[cr]),
                "val": np.ascontiguousarray(valq[cr]),
                "selu": np.ascontiguousarray(selu[cr]),
            }
        )

    trace = bool(int(os.environ.get("KERNEL_TRACE", "0")))
    res = None
    for attempt in range(3):
        try:
            res = bass_utils.run_bass_kernel_spmd(
                nc, in_maps, core_ids=list(range(NC)), trace=trace
            )
            break
        except Exception:
            if attempt == 2:
                raise
            import time as _time

            _time.sleep(10.0)
    global last_results
    last_results = res

    h2 = np.concatenate([res.results[cr]["out"] for cr in range(NC)], axis=0)[:N]
    out = np.zeros((pad_n_i, D), np.float32)
    out[pos_idx] = h2
    return out
